# revision 9
# baseline (speedup 1.0000x reference)
"""Trainium2 Bass kernel for nn_AggXPredictor (topk_masking).

Computes, for full inputs x[2048,256], t[2048,256], w[256,256] (all f32):
    r   = mean_o min(w[i,o], t[b,o])            # [B, IN]
    key = min(r[b,i], w[i,o])                   # [B, IN, OUT]
    ind = argmax_i key                          # [B, OUT]
    out = min(x[b, ind], w[ind, o])             # [B, OUT]

Sharding: data-parallel over batch across 8 cores (256 batches each), w
replicated.  Per core (layout [b-part, *-free] throughout):

  Step 1 (r): per (i, b-tile) one fused scalar_tensor_tensor:
      (t bypass) min wrow_bcast, accum_out = sequential-fp32 sum over o.
      The sequential accumulator bit-matches XLA's mean on this backend, so
      r (after an exact *2^-8 scale) is bit-identical to the reference's —
      making the final output bit-exact (no argmax near-tie flips).
  Step 2 (m): per (o, b-tile): key = tensor_tensor min(r, wrep_bcast);
      m = tensor_reduce max (order-insensitive, exact).
  Step 3 (extract, gather-free): [key==m] == [r>=m]*[w>=m]; ties are absent
      for these inputs, so scalar_tensor_tensor (key is_ge m) mult x|wrep
      with accum_out yields x[b,i*] and w[i*,o] exactly; out = min(xg, wg).

  w-row / wT-row broadcasts across partitions are DMAs with a
  partition-stride-0 DRAM source AP (wT staged to a DRAM scratch at setup).
"""

import numpy as np

import concourse.bass as bass
import concourse.tile as tile
from concourse import mybir
from concourse.bass_utils import run_bass_kernel_spmd
from concourse.masks import make_identity

F32 = mybir.dt.float32
OP = mybir.AluOpType

B, IN, OUT = 2048, 256, 256
NCORES = 8
BC = B // NCORES  # batches per core
P = 128

MAX_WAITS = 1


def _split_excess_waits(nc, max_waits=MAX_WAITS):
    """walrus in this env rejects instructions with >1 sync-wait; move
    excess waits onto preceding NoOps on the same engine."""
    n_split = 0
    for f in nc.m.functions:
        for bb in f.blocks:
            i = 0
            while i < len(bb.instructions):
                ins = bb.instructions[i]
                si = ins.sync_info
                if si is None:
                    i += 1
                    continue
                waits = list(si.on_wait)
                if len(waits) <= max_waits:
                    i += 1
                    continue
                si.on_wait = waits[:max_waits]
                extra = waits[max_waits:]
                k = 0
                while extra:
                    chunk, extra = extra[:max_waits], extra[max_waits:]
                    noop = mybir.InstNoOp(name=f"{ins.name}-wsplit-{k}")
                    noop.engine = ins.engine
                    noop.sync_info = mybir.SyncInfo(on_wait=chunk, on_update=[])
                    bb.instructions.insert(i, noop)
                    k += 1
                    i += 1
                    n_split += 1
                i += 1
    return n_split


def _bcast_row(row_ap):
    """Partition-stride-0 AP replicating a [1, F] DRAM row to 128 rows."""
    return bass.AP(
        tensor=row_ap.tensor,
        offset=row_ap.offset,
        ap=[[0, P]] + list(row_ap.ap[1:]),
    )


def build(split_waits=True):
    nc = bass.Bass(trn_type="TRN2")

    x_d = nc.dram_tensor("x", [BC, IN], F32, kind="ExternalInput")
    t_d = nc.dram_tensor("t", [BC, OUT], F32, kind="ExternalInput")
    w_d = nc.dram_tensor("w", [IN, OUT], F32, kind="ExternalInput")
    out_d = nc.dram_tensor("out", [BC, OUT], F32, kind="ExternalOutput")

    with tile.TileContext(nc) as tc:
        with (
            tc.tile_pool(name="consts", bufs=1) as consts,
            tc.tile_pool(name="inp", bufs=1) as inp,
            tc.tile_pool(name="wrow", bufs=4) as wrowp,
            tc.tile_pool(name="wrep", bufs=4) as wrepp,
            tc.tile_pool(name="key", bufs=3) as keyp,
            tc.tile_pool(name="junk", bufs=3) as junkp,
            tc.tile_pool(name="res", bufs=1) as resp,
            tc.tile_pool(name="outp", bufs=2) as outp,
            tc.tile_pool(name="dram", bufs=1, space="DRAM") as dramp,
        ):
            # ---------------- setup ----------------
            identity = consts.tile([P, P], F32)
            make_identity(nc, identity)

            x_sb = []  # [128b, 256i] per bt
            t_sb = []  # [128b, 256o] per bt
            w_sb = []  # [128i, 256o] per it
            for bt in range(2):
                xt_ = inp.tile([P, IN], F32, name=f"x{bt}", tag=f"x{bt}")
                nc.sync.dma_start(out=xt_, in_=x_d[bt * P:(bt + 1) * P, :])
                x_sb.append(xt_)
                tt_ = inp.tile([P, OUT], F32, name=f"t{bt}", tag=f"t{bt}")
                nc.sync.dma_start(out=tt_, in_=t_d[bt * P:(bt + 1) * P, :])
                t_sb.append(tt_)
                wt_ = inp.tile([P, OUT], F32, name=f"w{bt}", tag=f"w{bt}")
                nc.sync.dma_start(out=wt_, in_=w_d[bt * P:(bt + 1) * P, :])
                w_sb.append(wt_)

            # wT staged to DRAM scratch (for per-o row broadcasts)
            wT_dram = dramp.tile([OUT, IN], F32, name="wT_dram", tag="wT_dram")
            with tc.tile_pool(name="ps_tr", bufs=1, space="PSUM") as ps_tr:
                for ot in range(2):
                    wT_half = inp.tile([P, IN], F32, name=f"wT{ot}",
                                       tag=f"wT{ot}")
                    for it in range(2):
                        ptr = ps_tr.tile([P, P], F32, name="ptr", tag="ptr")
                        nc.tensor.transpose(
                            ptr, w_sb[it][:, ot * P:(ot + 1) * P], identity)
                        nc.scalar.copy(
                            out=wT_half[:, it * P:(it + 1) * P], in_=ptr)
                    nc.sync.dma_start(
                        out=wT_dram[ot * P:(ot + 1) * P, :], in_=wT_half)

            # ---------------- step 1: r ----------------
            rs_sb = [resp.tile([P, IN], F32, name=f"rs{bt}", tag=f"rs{bt}")
                     for bt in range(2)]
            for i in range(IN):
                wrow = wrowp.tile([P, OUT], F32, name="wrow", tag="wrow")
                nc.sync.dma_start(out=wrow, in_=_bcast_row(w_d[i:i + 1, :]))
                for bt in range(2):
                    junk = junkp.tile([P, OUT], F32, name="junk", tag="junk")
                    nc.vector.scalar_tensor_tensor(
                        out=junk,
                        in0=t_sb[bt],
                        scalar=0.0,
                        in1=wrow,
                        op0=OP.bypass,
                        op1=OP.min,
                        accum_out=rs_sb[bt][:, i:i + 1],
                    )

            r_sb = [resp.tile([P, IN], F32, name=f"r{bt}", tag=f"r{bt}")
                    for bt in range(2)]
            for bt in range(2):
                nc.scalar.mul(r_sb[bt], rs_sb[bt], 1.0 / 256.0)

            # ---------------- step 2+3: key/max/extract ----------------
            m_sb = [resp.tile([P, OUT], F32, name=f"m{bt}", tag=f"m{bt}")
                    for bt in range(2)]
            xg_sb = [resp.tile([P, OUT], F32, name=f"xg{bt}", tag=f"xg{bt}")
                     for bt in range(2)]
            wg_sb = [resp.tile([P, OUT], F32, name=f"wg{bt}", tag=f"wg{bt}")
                     for bt in range(2)]

            for o in range(OUT):
                wrep = wrepp.tile([P, IN], F32, name="wrep", tag="wrep")
                nc.sync.dma_start(out=wrep, in_=_bcast_row(wT_dram[o:o + 1, :]))
                for bt in range(2):
                    key = keyp.tile([P, IN], F32, name="key", tag="key")
                    nc.vector.tensor_tensor(key, r_sb[bt], wrep, OP.min)
                    nc.vector.tensor_reduce(
                        m_sb[bt][:, o:o + 1], key,
                        mybir.AxisListType.X, OP.max)
                    junk = junkp.tile([P, IN], F32, name="junk", tag="junk")
                    nc.vector.scalar_tensor_tensor(
                        out=junk,
                        in0=key,
                        scalar=m_sb[bt][:, o:o + 1],
                        in1=x_sb[bt],
                        op0=OP.is_ge,
                        op1=OP.mult,
                        accum_out=xg_sb[bt][:, o:o + 1],
                    )
                    junk2 = junkp.tile([P, IN], F32, name="junk", tag="junk")
                    nc.vector.scalar_tensor_tensor(
                        out=junk2,
                        in0=key,
                        scalar=m_sb[bt][:, o:o + 1],
                        in1=wrep,
                        op0=OP.is_ge,
                        op1=OP.mult,
                        accum_out=wg_sb[bt][:, o:o + 1],
                    )

            # ---------------- finalize ----------------
            for bt in range(2):
                outt = outp.tile([P, OUT], F32, name="outt", tag="outt")
                nc.vector.tensor_tensor(outt, xg_sb[bt], wg_sb[bt], OP.min)
                nc.sync.dma_start(
                    out=out_d[bt * P:(bt + 1) * P, :], in_=outt)

    if split_waits:
        _split_excess_waits(nc)
    return nc


_NC_CACHE = None


def _get_nc():
    global _NC_CACHE
    if _NC_CACHE is None:
        _NC_CACHE = build()
    return _NC_CACHE


def kernel(x: np.ndarray, t: np.ndarray, w: np.ndarray) -> np.ndarray:
    x = np.ascontiguousarray(np.asarray(x, dtype=np.float32))
    t = np.ascontiguousarray(np.asarray(t, dtype=np.float32))
    w = np.ascontiguousarray(np.asarray(w, dtype=np.float32))
    nc = _get_nc()
    in_maps = [
        {"x": x[c * BC:(c + 1) * BC], "t": t[c * BC:(c + 1) * BC], "w": w}
        for c in range(NCORES)
    ]
    res = run_bass_kernel_spmd(nc, in_maps, core_ids=list(range(NCORES)))
    return np.concatenate([res.results[c]["out"] for c in range(NCORES)], axis=0)


if __name__ == "__main__":
    rng = np.random.default_rng(0)
    out = kernel(
        rng.random((B, IN), dtype=np.float32),
        rng.random((B, OUT), dtype=np.float32),
        rng.random((IN, OUT), dtype=np.float32),
    )
    print(out.shape, out.dtype)
